# revision 28
# baseline (speedup 1.0000x reference)
"""Trainium2 Bass kernel for nn_AttentionModeEncoder (B=4, S=2048, HID=1024, 16 heads x 64).

Sharding: 8 cores = 4 batches x 2 head-groups (8 heads / 512 features per core).

Key ideas vs naive:
  - Masked keys (~50%) are dropped on the HOST: x_kv = x[mask==1] zero-padded to
    SK (multiple of 128).  Pad keys are neutralized by zeroing their V rows and
    denominator-ones entries (no exp bias needed), so scores/exp/AV all shrink ~2x.
  - Everything bf16 on the PE (1 cyc/row vs 4 for fp32), fp32 PSUM accumulate.
  - No PE transposes: x / x_kv transposed to [i, t] layout by the DMA XBAR
    (dma transpose, 2-byte dtype), weights pre-transposed on host, V^T moved
    into the [k, d] AV layout by DMA XBAR as well.
  - Head pairing: projections produce head pair (2j, 2j+1) in partition halves
    [0:64] / [64:128]; the two heads' score matmuls use disjoint PE row groups
    and run concurrently.  No K/Q duplication needed.
  - V bias via an augmented contraction row (bv x m_kv outer product) so pad
    rows of V are exactly zero.
  - Attention pipelined per (head-pair, q-chunk 512): scores (PE) -> exp
    (ScalarE, the only exp engine) -> AV (PE accumulate, with a ones column
    giving softmax denominators for free) -> PE-broadcast reciprocal normalize.
    Projections of the NEXT head pair are interleaved into the attention
    stream so the PE never idles while ScalarE works.
  - Phase C: y^T = Wo_part^T.T @ outT (bf16, fp32 accum + bias), streamed out.
Host sums the two partials per batch (cross-head-group reduction).
"""

import os
import sys
import numpy as np
from contextlib import ExitStack

for _p in ("/opt/trn_rl_repo", "/root/.axon_site/_ro/trn_rl_repo"):
    if os.path.isdir(_p) and _p not in sys.path:
        sys.path.insert(0, _p)

import concourse.bass as bass
import concourse.bacc as bacc
import concourse.mybir as mybir
import concourse.tile as tile

B, S, HID = 4, 2048, 1024
JC = 512                 # features per core (8 heads)
NCORES = 8
FP = mybir.dt.float32
BF = mybir.dt.bfloat16
MULT = mybir.AluOpType.mult

TRACE = False
LAST_RESULTS = {}


def build_nc(sk=1152):
    KT = sk // 128
    nc = bacc.Bacc()
    x = nc.declare_dram_parameter("x", [S, HID], BF, isOutput=False)
    xkv = nc.declare_dram_parameter("xkv", [sk, HID], BF, isOutput=False)
    mv = nc.declare_dram_parameter("mv", [sk], BF, isOutput=False)
    wqT = nc.declare_dram_parameter("wqT", [HID, JC], BF, isOutput=False)
    wkT = nc.declare_dram_parameter("wkT", [HID, JC], BF, isOutput=False)
    wvT = nc.declare_dram_parameter("wvT", [HID, JC], BF, isOutput=False)
    woT = nc.declare_dram_parameter("woT", [JC, HID], BF, isOutput=False)
    bqp = nc.declare_dram_parameter("bqp", [128, 4], FP, isOutput=False)
    bkp = nc.declare_dram_parameter("bkp", [128, 4], FP, isOutput=False)
    bvr = nc.declare_dram_parameter("bvr", [JC], BF, isOutput=False)
    bop = nc.declare_dram_parameter("bop", [128, 8], FP, isOutput=False)
    y = nc.declare_dram_parameter("y", [HID, S], FP, isOutput=True)

    # <=512-wide sub-chunks of the K-range, grouped into psum tiles of <=1024
    def chunks1024(total):
        out = []
        c0 = 0
        while c0 < total:
            clen = min(1024, total - c0)
            subs = []
            s0 = 0
            while s0 < clen:
                sl = min(512, clen - s0)
                subs.append((s0, sl))
                s0 += sl
            out.append((c0, clen, subs))
            c0 += clen
        return out

    with tile.TileContext(nc) as tc, ExitStack() as ctx:
        const = ctx.enter_context(tc.tile_pool(name="const", bufs=1))
        mid = ctx.enter_context(tc.tile_pool(name="mid", bufs=1))
        ptp = ctx.enter_context(tc.tile_pool(name="ptp", bufs=2))
        vst = ctx.enter_context(tc.tile_pool(name="vst", bufs=2))
        vtp = ctx.enter_context(tc.tile_pool(name="vtp", bufs=2))
        rp = ctx.enter_context(tc.tile_pool(name="rp", bufs=3))
        yp = ctx.enter_context(tc.tile_pool(name="yp", bufs=4))
        sp = ctx.enter_context(tc.tile_pool(name="sp", bufs=2, space="PSUM"))
        avp = ctx.enter_context(tc.tile_pool(name="avp", bufs=4, space="PSUM"))

        # ---------------- constants / weights into SBUF ----------------
        bq_s = const.tile([128, 4], FP)
        bk_s = const.tile([128, 4], FP)
        bo_s = const.tile([128, 8], FP)
        m_col = const.tile([128, KT], BF)
        m_row = const.tile([1, sk], BF)
        bv_row = const.tile([1, JC], BF)

        wq_s = mid.tile([128, 8, JC], BF)
        wk_s = mid.tile([128, 8, JC], BF)
        wv_s = mid.tile([128, 8, JC], BF)
        wo_s = mid.tile([128, 4, HID], BF)
        xT = mid.tile([128, 8, S], BF)         # [i, it, t]
        xkvT = mid.tile([128, 8, sk], BF)
        QTd = mid.tile([128, 4, S], BF)        # [d(2 heads), jt, q]
        KTd = mid.tile([128, 4, sk], BF)
        vts_tiles = {}                          # per-pair V^T staging [j, k]

        def vTs(jt):
            if jt not in vts_tiles:
                vts_tiles[jt] = vtp.tile([128, sk], BF, tag="vts",
                                         name=f"vts{jt}")
            return vts_tiles[jt]
        # [k, kt, head, d | 64x m]: cols 64:128 hold m_kv so the AV matmul
        # emits the softmax denominator ALREADY broadcast across rows 64:128.
        vaug = mid.tile([128, KT, 8, 128], BF)
        outT = mid.tile([128, 4, S], BF)       # attn out^T   [c, ct, t]

        # ALL DMAs go on the single sync queue: the XBAR transpose ucode is
        # corrupted by concurrently-executing regular DMAs (HW-observed), and
        # strict same-queue FIFO is the safe ordering.  Regular loads first,
        # then the transposes.
        wk_ap = wkT.rearrange("(it p) j -> p it j", p=128)
        nc.sync.dma_start(out=bk_s[:], in_=bkp[:, :])
        for it in range(8):
            # per-it wk chunks interleaved with the xkv transposes so the K
            # projection can start consuming it-slices almost immediately
            nc.sync.dma_start(out=wk_s[:, it, :], in_=wk_ap[:, it, :])
            nc.sync.dma_start(out=xkvT[:, it, :], in_=xkv[:, it * 128:(it + 1) * 128],
                              transpose=True)
        nc.sync.dma_start(out=wv_s[:], in_=wvT.rearrange("(it p) j -> p it j", p=128))
        nc.sync.dma_start(out=wq_s[:], in_=wqT.rearrange("(it p) j -> p it j", p=128))
        nc.sync.dma_start(out=bq_s[:], in_=bqp[:, :])
        nc.sync.dma_start(out=m_col[:], in_=mv.rearrange("(kt p) -> p kt", p=128))
        nc.sync.dma_start(out=m_row[:], in_=mv.rearrange("(a k) -> a k", a=1))
        nc.sync.dma_start(out=bv_row[:], in_=bvr.rearrange("(a j) -> a j", a=1))
        for it in range(8):
            nc.sync.dma_start(out=xT[:, it, :], in_=x[:, it * 128:(it + 1) * 128],
                              transpose=True)

        kchunks = chunks1024(sk)

        # ---------------- phase A piece emitters ----------------
        def q_proj_piece(jt, half):
            ps = sp.tile([128, 1024], FP, tag="ps")
            t0 = half * 1024
            for it in range(8):
                lw = wq_s[:, it, jt * 128:(jt + 1) * 128]
                nc.tensor.matmul(ps[:, 0:512], lhsT=lw, rhs=xT[:, it, t0:t0 + 512],
                                 start=(it == 0), stop=(it == 7),
                                 skip_group_check=True)
                nc.tensor.matmul(ps[:, 512:1024], lhsT=lw,
                                 rhs=xT[:, it, t0 + 512:t0 + 1024],
                                 start=(it == 0), stop=(it == 7),
                                 skip_group_check=True)
            nc.vector.tensor_scalar_add(
                QTd[:, jt, t0:t0 + 1024], ps[:], bq_s[:, jt:jt + 1])

        def k_proj_piece(jt, ci):
            c0, clen, subs = kchunks[ci]
            ps = sp.tile([128, 1024], FP, tag="ps")
            for it in range(8):
                lw = wk_s[:, it, jt * 128:(jt + 1) * 128]
                for s0, sl in subs:
                    nc.tensor.matmul(ps[:, s0:s0 + sl], lhsT=lw,
                                     rhs=xkvT[:, it, c0 + s0:c0 + s0 + sl],
                                     start=(it == 0), stop=(it == 7),
                                     skip_group_check=True)
            nc.vector.tensor_scalar_add(
                KTd[:, jt, c0:c0 + clen], ps[:, 0:clen], bk_s[:, jt:jt + 1])

        def v_proj_piece(jt, ci):
            c0, clen, subs = kchunks[ci]
            ps = sp.tile([128, 1024], FP, tag="ps")
            for it in range(8):
                lw = wv_s[:, it, jt * 128:(jt + 1) * 128]
                for s0, sl in subs:
                    nc.tensor.matmul(ps[:, s0:s0 + sl], lhsT=lw,
                                     rhs=xkvT[:, it, c0 + s0:c0 + s0 + sl],
                                     start=(it == 0), stop=False,
                                     skip_group_check=True)
            # bias as outer product bv[j] * m_kv[k]: zero on pad rows
            for s0, sl in subs:
                nc.tensor.matmul(ps[:, s0:s0 + sl],
                                 lhsT=bv_row[0:1, jt * 128:(jt + 1) * 128],
                                 rhs=m_row[0:1, c0 + s0:c0 + s0 + sl],
                                 start=False, stop=True, skip_group_check=True)
            nc.vector.tensor_copy(out=vTs(jt)[:, c0:c0 + clen], in_=ps[:, 0:clen])

        def v_finish_piece(jt):
            # V^T -> [k, kt, d] via DMA XBAR into a CONTIGUOUS staging tile
            # (the HW transpose ucode ignores strided dest APs), then DVE
            # scatters into vaug.  ones col = m_kv.
            for hh in range(2):
                h = 2 * jt + hh
                stg = vst.tile([128, KT, 64], BF, tag="stg")
                nc.sync.dma_start(out=stg[:],
                                  in_=vTs(jt)[hh * 64:hh * 64 + 64, :],
                                  transpose=True)
                nc.vector.tensor_copy(out=vaug[:, :, h, 0:64], in_=stg[:])
            for kt in range(KT):
                for hh in range(2):
                    nc.gpsimd.tensor_copy(
                        out=vaug[:, kt, 2 * jt + hh, 64:128],
                        in_=m_col[:, kt:kt + 1].broadcast_to((128, 64)))

        # ---------------- attention ----------------
        def attention_qc(hp, qc, todo):
            """One q-chunk of 512 for head pair hp; drains `todo` pieces
            interleaved between beats.  Normalize is pure DVE (the AV matmul
            already produced denominators broadcast in rows 64:128), so it
            never blocks the PE queue."""
            q0 = qc * 512
            PTt = ptp.tile([128, KT, 1024], BF, tag="PT")
            avA = avp.tile([128, 512], FP, tag="av", name="avA")
            avB = avp.tile([128, 512], FP, tag="av", name="avB")
            for kt in range(KT):
                k0 = kt * 128
                sct = sp.tile([128, 1024], FP, tag="ps", name="sc")
                nc.tensor.matmul(
                    sct[:, 0:512],
                    lhsT=KTd[0:64, hp, k0:k0 + 128],
                    rhs=QTd[0:64, hp, q0:q0 + 512],
                    start=True, stop=True, skip_group_check=True)
                nc.tensor.matmul(
                    sct[:, 512:1024],
                    lhsT=KTd[64:128, hp, k0:k0 + 128],
                    rhs=QTd[64:128, hp, q0:q0 + 512],
                    start=True, stop=True, skip_group_check=True)
                nc.scalar.activation(
                    PTt[:, kt, :], sct[:],
                    mybir.ActivationFunctionType.Exp, scale=0.125)
                for hh, av in ((0, avA), (1, avB)):
                    nc.tensor.matmul(
                        av[:, :],
                        lhsT=vaug[:, kt, 2 * hp + hh, :],
                        rhs=PTt[:, kt, hh * 512:(hh + 1) * 512],
                        start=(kt == 0), stop=(kt == KT - 1),
                        skip_group_check=True)
                if kt % 2 == 1 and todo:
                    todo.pop(0)()
            for hh, av in ((0, avA), (1, avB)):
                # reciprocal_approx_fast misreads PSUM at base partition 64
                # on HW; stage the denominator block to a base-0 SBUF tile.
                den = rp.tile([64, 512], FP, tag="den")
                nc.vector.tensor_copy(out=den[:], in_=av[64:128, :])
                recb = rp.tile([64, 512], FP, tag="recb")
                nc.vector.reciprocal_approx_fast(recb[:], den[:])
                nc.vector.tensor_tensor(
                    outT[hh * 64:hh * 64 + 64, hp, q0:q0 + 512],
                    av[0:64, :], recb[:], MULT)

        def phase_c_piece(tc, ot):
            # one output-tile of y^T[:, tc-chunk] = Wo_part^T.T @ outT + bo
            t0 = tc * 512
            ps = sp.tile([128, 1024], FP, tag="ps", name=f"c{ot}")
            for ct in range(4):
                nc.tensor.matmul(ps[:, 0:512],
                                 lhsT=wo_s[:, ct, ot * 128:(ot + 1) * 128],
                                 rhs=outT[:, ct, t0:t0 + 512],
                                 start=(ct == 0), stop=(ct == 3),
                                 skip_group_check=True)
            yt = yp.tile([128, 512], FP, tag="yt")
            nc.vector.tensor_scalar_add(yt[:], ps[:, 0:512],
                                        bo_s[:, ot:ot + 1])
            nc.sync.dma_start(
                out=y[ot * 128:(ot + 1) * 128, t0:t0 + 512], in_=yt[:])

        # ---------------- program order ----------------
        # upfront: all K projections (xkvT lands first), then V/Q for pair 0;
        # V/Q of later pairs + phase-C chunks interleave into attention.
        for jt in range(4):
            for ci in range(len(kchunks)):
                k_proj_piece(jt, ci)
        for ci in range(len(kchunks)):
            v_proj_piece(0, ci)
        v_finish_piece(0)
        q_proj_piece(0, 0)
        q_proj_piece(0, 1)
        # wo/bo queued AFTER the upfront stg transposes so attention's first
        # AV isn't gated on them
        nc.sync.dma_start(out=bo_s[:], in_=bop[:, :])
        nc.sync.dma_start(out=wo_s[:], in_=woT.rearrange("(ct p) o -> p ct o", p=128))
        todo = []
        for hp in range(4):
            if hp + 1 < 4:
                jn = hp + 1
                for ci in range(len(kchunks)):
                    todo.append(lambda ci=ci, jn=jn: v_proj_piece(jn, ci))
                todo.append(lambda jn=jn: v_finish_piece(jn))
                todo.append(lambda jn=jn: q_proj_piece(jn, 0))
                todo.append(lambda jn=jn: q_proj_piece(jn, 1))
            for qc in range(4):
                attention_qc(hp, qc, todo)
                if hp == 3:
                    todo.extend(
                        (lambda qc=qc, ot=ot: phase_c_piece(qc, ot))
                        for ot in range(8))
        while todo:
            todo.pop(0)()
    return nc


_NCS = {}


def _get_nc(sk):
    if sk not in _NCS:
        nc = build_nc(sk)
        nc.finalize()
        _NCS[sk] = nc
    return _NCS[sk]


def make_in_maps(x, mask, Wq, bq, Wk, bk, Wv, bv, Wo, bo):
    import ml_dtypes
    bf = ml_dtypes.bfloat16
    x = np.asarray(x, np.float32)
    mask = np.asarray(mask)
    counts = [int(mask[b].sum()) for b in range(B)]
    sk = max(128, ((max(counts) + 127) // 128) * 128)

    f32 = lambda a: np.ascontiguousarray(np.asarray(a, dtype=np.float32))
    xb = [np.ascontiguousarray(x[b].astype(bf)) for b in range(B)]
    xkvb = []
    mvb = []
    for b in range(B):
        idx = np.nonzero(mask[b])[0]
        xkv = np.zeros((sk, HID), bf)
        xkv[:len(idx)] = x[b][idx].astype(bf)
        m = np.zeros((sk,), bf)
        m[:len(idx)] = 1
        xkvb.append(xkv)
        mvb.append(m)
    gmaps = []
    for g in range(2):
        sl = slice(g * JC, (g + 1) * JC)
        gmaps.append({
            "wqT": np.ascontiguousarray(f32(Wq)[sl].T.astype(bf)),
            "wkT": np.ascontiguousarray(f32(Wk)[sl].T.astype(bf)),
            "wvT": np.ascontiguousarray(f32(Wv)[sl].T.astype(bf)),
            "woT": np.ascontiguousarray(f32(Wo)[:, sl].T.astype(bf)),
            "bqp": np.ascontiguousarray(f32(bq)[sl].reshape(4, 128).T),
            "bkp": np.ascontiguousarray(f32(bk)[sl].reshape(4, 128).T),
            "bvr": np.ascontiguousarray(f32(bv)[sl].astype(bf)),
            "bop": np.ascontiguousarray(
                (f32(bo) if g == 0 else np.zeros(HID, np.float32))
                .reshape(8, 128).T),
        })
    in_maps = []
    for c in range(NCORES):
        b, g = c // 2, c % 2
        in_maps.append({"x": xb[b], "xkv": xkvb[b], "mv": mvb[b], **gmaps[g]})
    return in_maps, sk


def kernel(x, mask, Wq, bq, Wk, bk, Wv, bv, Wo, bo):
    from concourse.bass_utils import run_bass_kernel_spmd

    in_maps, sk = make_in_maps(x, mask, Wq, bq, Wk, bk, Wv, bv, Wo, bo)
    nc = _get_nc(sk)
    kw = {}
    if TRACE:
        os.makedirs("/root/problem/trace_out", exist_ok=True)
        kw = dict(tmpdir="/root/problem/trace_out")
    r = run_bass_kernel_spmd(nc, in_maps, list(range(NCORES)), trace=TRACE, **kw)
    LAST_RESULTS["exec_time_ns"] = r.exec_time_ns
    LAST_RESULTS["mean_exec_time_ns"] = r.mean_exec_time_ns
    y = np.empty((B, S, HID), np.float32)
    for b in range(B):
        y[b] = (r.results[2 * b]["y"] + r.results[2 * b + 1]["y"]).T
    return y
